# revision 2
# baseline (speedup 1.0000x reference)
"""Grok1 MoE (unfused) Trainium2 Bass kernel — sparse top-2 routing.

Expert-parallel over 8 NeuronCores: core e owns expert e's w1/w3/w2.
Only top-2 of 8 experts matter per token, so each core processes just
the ~T*2/8 tokens routed to its expert instead of all T (4x fewer
matmul FLOPs than the dense formulation).

Host: computes the (tiny) router in fp32, gathers each expert's tokens
into a padded [H, C] slab, and scatter-adds the per-expert outputs back
into the full [T, H] result.
Device (per core): y = (gelu(x@w1.T) * (x@w3.T)) @ w2.T on its C-token
slab, scaled by the combine weight, fp16 matmuls with fp32 PSUM.

All device tensors are in [feature, token] (transposed) layout so the
matmul contraction dim always sits on SBUF partitions.
"""

import numpy as np

import concourse.bass as bass
import concourse.mybir as mybir
import concourse.tile as tile
from concourse import bacc
from concourse.bass import ts
from concourse.bass_utils import run_bass_kernel_spmd

T, H, F, E = 2048, 1024, 4096, 8
NCORES = 8
HC = H // 128   # 8 h-chunks
FC = F // 128   # 32 f-chunks

f32 = mybir.dt.float32
f16 = mybir.dt.float16

_CACHE = {}


def build_nc(C):
    """Sparse expert MLP over a padded C-token slab (C multiple of 128)."""
    assert C % 128 == 0 and C <= T
    # free-dim blocks for PSUM tiles (one 2KB bank = 512 fp32)
    blks = [(o, min(512, C - o)) for o in range(0, C, 512)]

    nc = bacc.Bacc(
        "TRN2",
        target_bir_lowering=False,
        debug=False,
        num_devices=NCORES,
    )

    xg = nc.dram_tensor("xg", [H, C], f16, kind="ExternalInput")
    cb = nc.dram_tensor("cb", [C], f32, kind="ExternalInput")
    w1t = nc.dram_tensor("w1t", [H, F], f16, kind="ExternalInput")
    w3t = nc.dram_tensor("w3t", [H, F], f16, kind="ExternalInput")
    w2t = nc.dram_tensor("w2t", [F, H], f16, kind="ExternalInput")
    out = nc.dram_tensor("out", [H, C], f32, kind="ExternalOutput")

    AF = mybir.ActivationFunctionType

    with tile.TileContext(nc) as tc:
        with (
            tc.tile_pool(name="big", bufs=1) as big,
            tc.tile_pool(name="singles", bufs=1) as singles,
            tc.tile_pool(name="wpool", bufs=3) as wpool,
            tc.tile_pool(name="w2pool", bufs=2) as w2pool,
            tc.tile_pool(name="evict", bufs=3) as evict,
            tc.tile_pool(name="psum_gu", bufs=2, space="PSUM") as psum_gu,
            tc.tile_pool(name="psum_o", bufs=2, space="PSUM") as psum_o,
        ):
            # ---- load inputs ----
            x_sb = singles.tile([128, HC, C], f16)
            nc.sync.dma_start(
                out=x_sb, in_=xg.ap().rearrange("(c p) t -> p c t", p=128)
            )
            cb_b = singles.tile([128, C], f32)
            cb_src = bass.AP(tensor=cb.ap().tensor, offset=0, ap=[[0, 128], [1, C]])
            nc.sync.dma_start(out=cb_b, in_=cb_src)

            # ---- phase 1: gus = gelu(w1 @ x) * (w3 @ x), [F-part, tokens] ----
            gus = big.tile([128, FC, C], f16)
            for f in range(FC):
                w1f = wpool.tile([128, HC, 128], f16, tag="w1f")
                nc.sync.dma_start(
                    out=w1f,
                    in_=w1t.ap()[:, ts(f, 128)].rearrange("(c p) m -> p c m", p=128),
                )
                w3f = wpool.tile([128, HC, 128], f16, tag="w3f")
                nc.sync.dma_start(
                    out=w3f,
                    in_=w3t.ap()[:, ts(f, 128)].rearrange("(c p) m -> p c m", p=128),
                )
                for off, sz in blks:
                    g_ps = psum_gu.tile([128, 512], f32, tag="g")
                    for h in range(HC):
                        nc.tensor.matmul(
                            g_ps[:, :sz],
                            lhsT=w1f[:, h, :],
                            rhs=x_sb[:, h, off : off + sz],
                            start=(h == 0),
                            stop=(h == HC - 1),
                        )
                    u_ps = psum_gu.tile([128, 512], f32, tag="u")
                    for h in range(HC):
                        nc.tensor.matmul(
                            u_ps[:, :sz],
                            lhsT=w3f[:, h, :],
                            rhs=x_sb[:, h, off : off + sz],
                            start=(h == 0),
                            stop=(h == HC - 1),
                        )
                    gs = evict.tile([128, 512], f32, tag="gs")
                    nc.scalar.activation(gs[:, :sz], g_ps[:, :sz], AF.Gelu)
                    nc.vector.tensor_mul(
                        gus[:, f, off : off + sz], gs[:, :sz], u_ps[:, :sz]
                    )

            # ---- phase 2: out[h,:] = (w2 @ gus) * cb ----
            for h in range(HC):
                halves = []
                for hv in range(2):
                    w2h = w2pool.tile([128, FC // 2, 128], f16, tag="w2h")
                    nc.sync.dma_start(
                        out=w2h,
                        in_=w2t.ap()[ts(hv, F // 2), ts(h, 128)].rearrange(
                            "(c p) m -> p c m", p=128
                        ),
                    )
                    halves.append(w2h)
                for off, sz in blks:
                    o_ps = psum_o.tile([128, 512], f32, tag="o")
                    for f in range(FC):
                        nc.tensor.matmul(
                            o_ps[:, :sz],
                            lhsT=halves[f // (FC // 2)][:, f % (FC // 2), :],
                            rhs=gus[:, f, off : off + sz],
                            start=(f == 0),
                            stop=(f == FC - 1),
                        )
                    o_sb = evict.tile([128, 512], f32, tag="osb")
                    nc.vector.tensor_mul(
                        o_sb[:, :sz], o_ps[:, :sz], cb_b[:, off : off + sz]
                    )
                    nc.sync.dma_start(
                        out=out.ap()[ts(h, 128), off : off + sz], in_=o_sb[:, :sz]
                    )
    nc.finalize()
    return nc


def _route(hidden_states, gate_w):
    """Exact fp32 router matching the reference: softcap -> softmax -> top2."""
    hs = hidden_states.astype(np.float32)
    logits = hs @ gate_w.T.astype(np.float32)
    logits = 30.0 * np.tanh(logits / 30.0)
    lmax = logits.max(axis=-1, keepdims=True)
    p = np.exp(logits - lmax)
    probs = p / p.sum(axis=-1, keepdims=True)
    idx = np.argsort(-probs, axis=-1, kind="stable")[:, :2]
    vals = np.take_along_axis(probs, idx, axis=-1)
    return idx, vals


def kernel(hidden_states, gate_w, w1, w2, w3, trace=False):
    hidden_states = np.asarray(hidden_states, dtype=np.float32)
    gate_w = np.asarray(gate_w, dtype=np.float32)
    w1 = np.asarray(w1, dtype=np.float32)
    w2 = np.asarray(w2, dtype=np.float32)
    w3 = np.asarray(w3, dtype=np.float32)

    idx, vals = _route(hidden_states, gate_w)
    toks = []
    cvals = []
    for e in range(E):
        hit = idx == e                                     # [T, 2]
        tok_e = np.where(hit.any(axis=1))[0]
        toks.append(tok_e)
        cvals.append((vals * hit)[tok_e].sum(axis=1).astype(np.float32))
    nmax = max(len(t) for t in toks)
    C = max(128, -(-nmax // 128) * 128)

    if C not in _CACHE:
        _CACHE[C] = build_nc(C)
    nc = _CACHE[C]

    xT16 = np.ascontiguousarray(hidden_states.T).astype(np.float16)  # [H, T]
    in_maps = []
    for e in range(NCORES):
        n_e = len(toks[e])
        xg = np.zeros((H, C), dtype=np.float16)
        xg[:, :n_e] = xT16[:, toks[e]]
        cbv = np.zeros((C,), dtype=np.float32)
        cbv[:n_e] = cvals[e]
        in_maps.append(
            {
                "xg": xg,
                "cb": cbv,
                "w1t": np.ascontiguousarray(w1[e].T).astype(np.float16),
                "w3t": np.ascontiguousarray(w3[e].T).astype(np.float16),
                "w2t": np.ascontiguousarray(w2[e].T).astype(np.float16),
            }
        )

    res = run_bass_kernel_spmd(nc, in_maps, core_ids=list(range(NCORES)), trace=trace)
    out = np.zeros((T, H), dtype=np.float32)
    for e in range(NCORES):
        n_e = len(toks[e])
        out[toks[e]] += res.results[e]["out"][:, :n_e].T
    _CACHE["last_results"] = res
    return out


if __name__ == "__main__":
    rng = np.random.default_rng(0)
    hs = rng.standard_normal((T, H), dtype=np.float32)
    gw = (rng.standard_normal((E, H)) * 0.02).astype(np.float32)
    w1 = (rng.standard_normal((E, F, H)) * 0.02).astype(np.float32)
    w2 = (rng.standard_normal((E, H, F)) * 0.02).astype(np.float32)
    w3 = (rng.standard_normal((E, F, H)) * 0.02).astype(np.float32)
    out = kernel(hs, gw, w1, w2, w3)
    print("out", out.shape, out.dtype, np.abs(out).max())


# revision 3
# speedup vs baseline: 1.0214x; 1.0214x over previous
"""Grok1 MoE (unfused) Trainium2 Bass kernel — sparse top-2 routing.

Expert-parallel over 8 NeuronCores: core e owns expert e's w1/w3/w2.
Only top-2 of 8 experts matter per token, so each core processes just
the ~T*2/8 tokens routed to its expert instead of all T (4x fewer
matmul FLOPs than the dense formulation).

Host: computes the (tiny) router in fp32, gathers each expert's tokens
into a padded [128, HC, C] slab, and scatter-adds the per-expert outputs
back into the full [T, H] result.
Device (per core): y = (gelu(x@w1.T) * (x@w3.T)) @ w2.T on its C-token
slab, scaled by the combine weight, fp16 matmuls with fp32 PSUM.

All weights are pre-transposed on the host into the exact SBUF tile
layout so every weight DMA is a single fully-contiguous block (the
naive [H, F] layout makes the DMA read 256B strided lines and caps
effective HBM bandwidth well below peak).
"""

import numpy as np

import concourse.bass as bass
import concourse.mybir as mybir
import concourse.tile as tile
from concourse import bacc
from concourse.bass_utils import run_bass_kernel_spmd

T, H, F, E = 2048, 1024, 4096, 8
NCORES = 8
HC = H // 128   # 8 h-chunks
FC = F // 128   # 32 f-chunks

f32 = mybir.dt.float32
f16 = mybir.dt.float16

_CACHE = {}


def build_nc(C):
    """Sparse expert MLP over a padded C-token slab (C multiple of 128)."""
    assert C % 128 == 0 and C <= T
    # free-dim blocks for PSUM tiles (one 2KB bank = 512 fp32)
    blks = [(o, min(512, C - o)) for o in range(0, C, 512)]

    nc = bacc.Bacc(
        "TRN2",
        target_bir_lowering=False,
        debug=False,
        num_devices=NCORES,
    )

    xg = nc.dram_tensor("xg", [128, HC, C], f16, kind="ExternalInput")
    cb = nc.dram_tensor("cb", [C], f32, kind="ExternalInput")
    w1p = nc.dram_tensor("w1p", [FC, 128, HC, 128], f16, kind="ExternalInput")
    w3p = nc.dram_tensor("w3p", [FC, 128, HC, 128], f16, kind="ExternalInput")
    w2p = nc.dram_tensor("w2p", [HC, 2, 128, FC // 2, 128], f16, kind="ExternalInput")
    out = nc.dram_tensor("out", [H, C], f32, kind="ExternalOutput")

    AF = mybir.ActivationFunctionType

    with tile.TileContext(nc) as tc:
        with (
            tc.tile_pool(name="big", bufs=1) as big,
            tc.tile_pool(name="singles", bufs=1) as singles,
            tc.tile_pool(name="wpool", bufs=3) as wpool,
            tc.tile_pool(name="w2pool", bufs=2) as w2pool,
            tc.tile_pool(name="evict", bufs=3) as evict,
            tc.tile_pool(name="psum_gu", bufs=2, space="PSUM") as psum_gu,
            tc.tile_pool(name="psum_o", bufs=2, space="PSUM") as psum_o,
        ):
            # ---- load inputs (already in SBUF layout; contiguous DMAs) ----
            x_sb = singles.tile([128, HC, C], f16)
            nc.sync.dma_start(out=x_sb, in_=xg.ap())
            cb_b = singles.tile([128, C], f32)
            cb_src = bass.AP(tensor=cb.ap().tensor, offset=0, ap=[[0, 128], [1, C]])
            nc.sync.dma_start(out=cb_b, in_=cb_src)

            # ---- phase 1: gus = gelu(w1 @ x) * (w3 @ x), [F-part, tokens] ----
            gus = big.tile([128, FC, C], f16)
            for f in range(FC):
                w1f = wpool.tile([128, HC, 128], f16, tag="w1f")
                nc.sync.dma_start(out=w1f, in_=w1p.ap()[f])
                w3f = wpool.tile([128, HC, 128], f16, tag="w3f")
                nc.sync.dma_start(out=w3f, in_=w3p.ap()[f])
                for off, sz in blks:
                    g_ps = psum_gu.tile([128, 512], f32, tag="g")
                    for h in range(HC):
                        nc.tensor.matmul(
                            g_ps[:, :sz],
                            lhsT=w1f[:, h, :],
                            rhs=x_sb[:, h, off : off + sz],
                            start=(h == 0),
                            stop=(h == HC - 1),
                        )
                    u_ps = psum_gu.tile([128, 512], f32, tag="u")
                    for h in range(HC):
                        nc.tensor.matmul(
                            u_ps[:, :sz],
                            lhsT=w3f[:, h, :],
                            rhs=x_sb[:, h, off : off + sz],
                            start=(h == 0),
                            stop=(h == HC - 1),
                        )
                    gs = evict.tile([128, 512], f32, tag="gs")
                    nc.scalar.activation(gs[:, :sz], g_ps[:, :sz], AF.Gelu)
                    nc.vector.tensor_mul(
                        gus[:, f, off : off + sz], gs[:, :sz], u_ps[:, :sz]
                    )

            # ---- phase 2: out[h,:] = (w2 @ gus) * cb ----
            for h in range(HC):
                halves = []
                for hv in range(2):
                    w2h = w2pool.tile([128, FC // 2, 128], f16, tag="w2h")
                    nc.sync.dma_start(out=w2h, in_=w2p.ap()[h, hv])
                    halves.append(w2h)
                for off, sz in blks:
                    o_ps = psum_o.tile([128, 512], f32, tag="o")
                    for f in range(FC):
                        nc.tensor.matmul(
                            o_ps[:, :sz],
                            lhsT=halves[f // (FC // 2)][:, f % (FC // 2), :],
                            rhs=gus[:, f, off : off + sz],
                            start=(f == 0),
                            stop=(f == FC - 1),
                        )
                    o_sb = evict.tile([128, 512], f32, tag="osb")
                    nc.vector.tensor_mul(
                        o_sb[:, :sz], o_ps[:, :sz], cb_b[:, off : off + sz]
                    )
                    nc.sync.dma_start(
                        out=out.ap()[bass.ts(h, 128), off : off + sz],
                        in_=o_sb[:, :sz],
                    )
    nc.finalize()
    return nc


def _route(hidden_states, gate_w):
    """Exact fp32 router matching the reference: softcap -> softmax -> top2."""
    hs = hidden_states.astype(np.float32)
    logits = hs @ gate_w.T.astype(np.float32)
    logits = 30.0 * np.tanh(logits / 30.0)
    lmax = logits.max(axis=-1, keepdims=True)
    p = np.exp(logits - lmax)
    probs = p / p.sum(axis=-1, keepdims=True)
    idx = np.argsort(-probs, axis=-1, kind="stable")[:, :2]
    vals = np.take_along_axis(probs, idx, axis=-1)
    return idx, vals


def _prep_weights(w1, w2, w3):
    """Pre-transpose expert weights into contiguous SBUF tile layouts."""
    wmaps = []
    for e in range(NCORES):
        # w1p[f, p, c, m] = w1[e][f*128+m, c*128+p]
        w1p = np.ascontiguousarray(
            w1[e].astype(np.float16).reshape(FC, 128, HC, 128).transpose(0, 3, 2, 1)
        )
        w3p = np.ascontiguousarray(
            w3[e].astype(np.float16).reshape(FC, 128, HC, 128).transpose(0, 3, 2, 1)
        )
        # w2p[h, hv, p, c, m] = w2[e][h*128+m, hv*F/2 + c*128+p]
        w2p = np.ascontiguousarray(
            w2[e]
            .astype(np.float16)
            .reshape(HC, 128, 2, FC // 2, 128)
            .transpose(0, 2, 4, 3, 1)
        )
        wmaps.append({"w1p": w1p, "w3p": w3p, "w2p": w2p})
    return wmaps


def kernel(hidden_states, gate_w, w1, w2, w3, trace=False):
    hidden_states = np.asarray(hidden_states, dtype=np.float32)
    gate_w = np.asarray(gate_w, dtype=np.float32)
    w1 = np.asarray(w1, dtype=np.float32)
    w2 = np.asarray(w2, dtype=np.float32)
    w3 = np.asarray(w3, dtype=np.float32)

    idx, vals = _route(hidden_states, gate_w)
    toks = []
    cvals = []
    for e in range(E):
        hit = idx == e                                     # [T, 2]
        tok_e = np.where(hit.any(axis=1))[0]
        toks.append(tok_e)
        cvals.append((vals * hit)[tok_e].sum(axis=1).astype(np.float32))
    nmax = max(len(t) for t in toks)
    C = max(128, -(-nmax // 128) * 128)

    if C not in _CACHE:
        _CACHE[C] = build_nc(C)
    nc = _CACHE[C]

    xT16 = np.ascontiguousarray(hidden_states.T).astype(np.float16)  # [H, T]
    wmaps = _prep_weights(w1, w2, w3)
    in_maps = []
    for e in range(NCORES):
        n_e = len(toks[e])
        xg = np.zeros((128, HC, C), dtype=np.float16)
        # xg[p, c, :n] = x[c*128+p, toks]
        xg[:, :, :n_e] = xT16.reshape(HC, 128, T)[:, :, toks[e]].transpose(1, 0, 2)
        cbv = np.zeros((C,), dtype=np.float32)
        cbv[:n_e] = cvals[e]
        in_maps.append({"xg": xg, "cb": cbv, **wmaps[e]})

    res = run_bass_kernel_spmd(nc, in_maps, core_ids=list(range(NCORES)), trace=trace)
    out = np.zeros((T, H), dtype=np.float32)
    for e in range(NCORES):
        n_e = len(toks[e])
        out[toks[e]] += res.results[e]["out"][:, :n_e].T
    _CACHE["last_results"] = res
    return out


if __name__ == "__main__":
    rng = np.random.default_rng(0)
    hs = rng.standard_normal((T, H), dtype=np.float32)
    gw = (rng.standard_normal((E, H)) * 0.02).astype(np.float32)
    w1 = (rng.standard_normal((E, F, H)) * 0.02).astype(np.float32)
    w2 = (rng.standard_normal((E, H, F)) * 0.02).astype(np.float32)
    w3 = (rng.standard_normal((E, F, H)) * 0.02).astype(np.float32)
    out = kernel(hs, gw, w1, w2, w3)
    print("out", out.shape, out.dtype, np.abs(out).max())


# revision 4
# speedup vs baseline: 1.3025x; 1.2751x over previous
"""Grok1 MoE (unfused) Trainium2 Bass kernel — sparse top-2 routing.

Expert-parallel over 8 NeuronCores: core e owns expert e's w1/w3/w2.
Only top-2 of 8 experts matter per token, so each core processes just
the ~T*2/8 tokens routed to its expert instead of all T (4x fewer
matmul FLOPs than the dense formulation).

Host: computes the (tiny) router in fp32, gathers each expert's tokens
into a padded token slab, and scatter-adds the per-expert outputs back
into the full [T, H] result.
Device (per core): y = (gelu(x@w1.T) * (x@w3.T)) @ w2.T on its C-token
slab, scaled by the combine weight, fp16 matmuls with fp32 PSUM.

Perf structure (from NTFF trace analysis):
- weights pre-transposed on host into exact SBUF tile layout so every
  weight DMA is one fully-contiguous block,
- w1/w3 stream just-in-time on the SP DMA ring (bufs=4 prefetch);
  ALL of w2 (64KB/partition) preloads on the Activation DMA ring
  during phase 1, so phase 2 never waits on DMA,
- token dim padded only to a multiple of 16 and split into near-equal
  PSUM-bank-sized blocks (no 128-wide tail matmuls),
- a few dummy warmup matmuls ramp the PE p-state while x loads.
"""

import numpy as np

import concourse.bass as bass
import concourse.mybir as mybir
import concourse.tile as tile
from concourse import bacc
from concourse.bass import ts
from concourse.bass_utils import run_bass_kernel_spmd

T, H, F, E = 2048, 1024, 4096, 8
NCORES = 8
HC = H // 128   # 8 h-chunks
FC = F // 128   # 32 f-chunks
NWARM = 24      # PE p-state warmup matmuls

f32 = mybir.dt.float32
f16 = mybir.dt.float16

_CACHE = {}


def _blocks(C):
    """Split C tokens into near-equal blocks of <=512 (PSUM bank limit),
    each a multiple of 16 except possibly the last."""
    nblk = -(-C // 512)
    base = -(-C // nblk)
    base = -(-base // 16) * 16
    blks = []
    off = 0
    while off < C:
        sz = min(base, C - off)
        blks.append((off, sz))
        off += sz
    return blks


def build_nc(C):
    assert C % 16 == 0 and C <= T
    blks = _blocks(C)

    nc = bacc.Bacc(
        "TRN2",
        target_bir_lowering=False,
        debug=False,
        num_devices=NCORES,
    )

    xgs = [
        nc.dram_tensor(f"xg{i}", [128, HC, sz], f16, kind="ExternalInput")
        for i, (_, sz) in enumerate(blks)
    ]
    cb = nc.dram_tensor("cb", [C], f32, kind="ExternalInput")
    w1p = nc.dram_tensor("w1p", [FC, 128, HC, 128], f16, kind="ExternalInput")
    w3p = nc.dram_tensor("w3p", [FC, 128, HC, 128], f16, kind="ExternalInput")
    w2p = nc.dram_tensor("w2p", [HC, 2, 128, FC // 2, 128], f16, kind="ExternalInput")
    out = nc.dram_tensor("out", [H, C], f32, kind="ExternalOutput")

    AF = mybir.ActivationFunctionType

    with tile.TileContext(nc) as tc:
        with (
            tc.tile_pool(name="big", bufs=1) as big,
            tc.tile_pool(name="singles", bufs=1) as singles,
            tc.tile_pool(name="wpool", bufs=4) as wpool,
            tc.tile_pool(name="w2pool", bufs=1) as w2pool,
            tc.tile_pool(name="evict", bufs=3) as evict,
            tc.tile_pool(name="psum_w", bufs=2, space="PSUM") as psum_w,
            tc.tile_pool(name="psum_gu", bufs=2, space="PSUM") as psum_gu,
            tc.tile_pool(name="psum_o", bufs=2, space="PSUM") as psum_o,
        ):
            # ---- PE warmup: ramp the p-state while inputs stream in ----
            warm = singles.tile([128, 512], f16, tag="warm")
            nc.vector.memset(warm, 0.0)
            for _ in range(NWARM):
                wps = psum_w.tile([128, 512], f32, tag="wm")
                nc.tensor.matmul(wps, lhsT=warm[:, :128], rhs=warm, start=True, stop=True)

            # ---- input loads ----
            # SP ring: x block 0, then the just-in-time w1/w3 stream.
            # ACT ring: x block 1+, cb, then the whole-w2 preload.
            xb = []
            for i, (_, sz) in enumerate(blks):
                t = singles.tile([128, HC, sz], f16, tag=f"xb{i}")
                eng = nc.sync if i == 0 else nc.scalar
                eng.dma_start(out=t, in_=xgs[i].ap())
                xb.append(t)
            cb_b = singles.tile([128, C], f32, tag="cb")
            cb_src = bass.AP(tensor=cb.ap().tensor, offset=0, ap=[[0, 128], [1, C]])
            nc.scalar.dma_start(out=cb_b, in_=cb_src)
            w2sb = {}
            for h in range(HC):
                for hv in range(2):
                    t = w2pool.tile([128, FC // 2, 128], f16, tag=f"w2_{h}_{hv}")
                    nc.scalar.dma_start(out=t, in_=w2p.ap()[h, hv])
                    w2sb[h, hv] = t

            # ---- phase 1: gus = gelu(w1 @ x) * (w3 @ x), [F-part, tokens] ----
            gus = big.tile([128, FC, C], f16)
            for f in range(FC):
                w1f = wpool.tile([128, HC, 128], f16, tag="w1f")
                nc.sync.dma_start(out=w1f, in_=w1p.ap()[f])
                w3f = wpool.tile([128, HC, 128], f16, tag="w3f")
                nc.sync.dma_start(out=w3f, in_=w3p.ap()[f])
                for bi, (off, sz) in enumerate(blks):
                    g_ps = psum_gu.tile([128, 512], f32, tag="g")
                    for h in range(HC):
                        nc.tensor.matmul(
                            g_ps[:, :sz],
                            lhsT=w1f[:, h, :],
                            rhs=xb[bi][:, h, :],
                            start=(h == 0),
                            stop=(h == HC - 1),
                        )
                    u_ps = psum_gu.tile([128, 512], f32, tag="u")
                    for h in range(HC):
                        nc.tensor.matmul(
                            u_ps[:, :sz],
                            lhsT=w3f[:, h, :],
                            rhs=xb[bi][:, h, :],
                            start=(h == 0),
                            stop=(h == HC - 1),
                        )
                    gs = evict.tile([128, 512], f32, tag="gs")
                    nc.scalar.activation(gs[:, :sz], g_ps[:, :sz], AF.Gelu)
                    nc.vector.tensor_mul(
                        gus[:, f, off : off + sz], gs[:, :sz], u_ps[:, :sz]
                    )

            # ---- phase 2: out[h,:] = (w2 @ gus) * cb (w2 already in SBUF) ----
            for h in range(HC):
                for off, sz in blks:
                    o_ps = psum_o.tile([128, 512], f32, tag="o")
                    for f in range(FC):
                        nc.tensor.matmul(
                            o_ps[:, :sz],
                            lhsT=w2sb[h, f // (FC // 2)][:, f % (FC // 2), :],
                            rhs=gus[:, f, off : off + sz],
                            start=(f == 0),
                            stop=(f == FC - 1),
                        )
                    o_sb = evict.tile([128, 512], f32, tag="osb")
                    nc.vector.tensor_mul(
                        o_sb[:, :sz], o_ps[:, :sz], cb_b[:, off : off + sz]
                    )
                    nc.sync.dma_start(
                        out=out.ap()[ts(h, 128), off : off + sz], in_=o_sb[:, :sz]
                    )
    nc.finalize()
    return nc


def _route(hidden_states, gate_w):
    """Exact fp32 router matching the reference: softcap -> softmax -> top2."""
    hs = hidden_states.astype(np.float32)
    logits = hs @ gate_w.T.astype(np.float32)
    logits = 30.0 * np.tanh(logits / 30.0)
    lmax = logits.max(axis=-1, keepdims=True)
    p = np.exp(logits - lmax)
    probs = p / p.sum(axis=-1, keepdims=True)
    idx = np.argsort(-probs, axis=-1, kind="stable")[:, :2]
    vals = np.take_along_axis(probs, idx, axis=-1)
    return idx, vals


def _prep_weights(w1, w2, w3):
    """Pre-transpose expert weights into contiguous SBUF tile layouts."""
    wmaps = []
    for e in range(NCORES):
        # w1p[f, p, c, m] = w1[e][f*128+m, c*128+p]
        w1p = np.ascontiguousarray(
            w1[e].astype(np.float16).reshape(FC, 128, HC, 128).transpose(0, 3, 2, 1)
        )
        w3p = np.ascontiguousarray(
            w3[e].astype(np.float16).reshape(FC, 128, HC, 128).transpose(0, 3, 2, 1)
        )
        # w2p[h, hv, p, c, m] = w2[e][h*128+m, hv*F/2 + c*128+p]
        w2p = np.ascontiguousarray(
            w2[e]
            .astype(np.float16)
            .reshape(HC, 128, 2, FC // 2, 128)
            .transpose(0, 2, 4, 3, 1)
        )
        wmaps.append({"w1p": w1p, "w3p": w3p, "w2p": w2p})
    return wmaps


def kernel(hidden_states, gate_w, w1, w2, w3, trace=False):
    hidden_states = np.asarray(hidden_states, dtype=np.float32)
    gate_w = np.asarray(gate_w, dtype=np.float32)
    w1 = np.asarray(w1, dtype=np.float32)
    w2 = np.asarray(w2, dtype=np.float32)
    w3 = np.asarray(w3, dtype=np.float32)

    idx, vals = _route(hidden_states, gate_w)
    toks = []
    cvals = []
    for e in range(E):
        hit = idx == e                                     # [T, 2]
        tok_e = np.where(hit.any(axis=1))[0]
        toks.append(tok_e)
        cvals.append((vals * hit)[tok_e].sum(axis=1).astype(np.float32))
    nmax = max(len(t) for t in toks)
    C = max(16, -(-nmax // 16) * 16)
    blks = _blocks(C)

    if C not in _CACHE:
        _CACHE[C] = build_nc(C)
    nc = _CACHE[C]

    xT16 = np.ascontiguousarray(hidden_states.T).astype(np.float16)  # [H, T]
    wmaps = _prep_weights(w1, w2, w3)
    in_maps = []
    for e in range(NCORES):
        n_e = len(toks[e])
        xg = np.zeros((128, HC, C), dtype=np.float16)
        # xg[p, c, :n] = x[c*128+p, toks]
        xg[:, :, :n_e] = xT16.reshape(HC, 128, T)[:, :, toks[e]].transpose(1, 0, 2)
        cbv = np.zeros((C,), dtype=np.float32)
        cbv[:n_e] = cvals[e]
        m = {"cb": cbv, **wmaps[e]}
        for i, (off, sz) in enumerate(blks):
            m[f"xg{i}"] = np.ascontiguousarray(xg[:, :, off : off + sz])
        in_maps.append(m)

    res = run_bass_kernel_spmd(nc, in_maps, core_ids=list(range(NCORES)), trace=trace)
    out = np.zeros((T, H), dtype=np.float32)
    for e in range(NCORES):
        n_e = len(toks[e])
        out[toks[e]] += res.results[e]["out"][:, :n_e].T
    _CACHE["last_results"] = res
    return out


if __name__ == "__main__":
    rng = np.random.default_rng(0)
    hs = rng.standard_normal((T, H), dtype=np.float32)
    gw = (rng.standard_normal((E, H)) * 0.02).astype(np.float32)
    w1 = (rng.standard_normal((E, F, H)) * 0.02).astype(np.float32)
    w2 = (rng.standard_normal((E, H, F)) * 0.02).astype(np.float32)
    w3 = (rng.standard_normal((E, F, H)) * 0.02).astype(np.float32)
    out = kernel(hs, gw, w1, w2, w3)
    print("out", out.shape, out.dtype, np.abs(out).max())


# revision 7
# speedup vs baseline: 1.3924x; 1.0691x over previous
"""Grok1 MoE (unfused) Trainium2 Bass kernel — sparse top-2 routing.

Expert-parallel over 8 NeuronCores: core e owns expert e's w1/w3/w2.
Only top-2 of 8 experts matter per token, so each core processes just
the ~T*2/8 tokens routed to its expert instead of all T (4x fewer
matmul FLOPs than the dense formulation).

Host: computes the (tiny) router in fp32, gathers each expert's tokens
into a padded token slab, and scatter-adds the per-expert outputs back
into the full [T, H] result.
Device (per core): y = (gelu(x@w1.T) * (x@w3.T)) @ w2.T on its C-token
slab, scaled by the combine weight, fp16 matmuls with fp32 PSUM.

Perf structure (from NTFF trace analysis):
- weights pre-transposed on host into exact SBUF tile layout so every
  weight DMA is one fully-contiguous block,
- w1/w3 stream just-in-time on the SP DMA ring (bufs=4 prefetch);
  ALL of w2 (64KB/partition) preloads on the Activation DMA ring
  during phase 1, so phase 2 never waits on DMA,
- token dim padded only to a multiple of 16 and split into near-equal
  PSUM-bank-sized blocks (no 128-wide tail matmuls),
- a few dummy warmup matmuls ramp the PE p-state while x loads.
"""

import numpy as np

import concourse.bass as bass
import concourse.mybir as mybir
import concourse.tile as tile
from concourse import bacc
from concourse.bass import ts
from concourse.bass_utils import run_bass_kernel_spmd

T, H, F, E = 2048, 1024, 4096, 8
NCORES = 8
HC = H // 128   # 8 h-chunks
FC = F // 128   # 32 f-chunks
NWARM = 14      # PE p-state warmup matmuls

f32 = mybir.dt.float32
f16 = mybir.dt.float16

_CACHE = {}


def _blocks(C):
    """Split C tokens into near-equal blocks of <=512 (PSUM bank limit),
    each a multiple of 16 except possibly the last."""
    nblk = -(-C // 512)
    base = -(-C // nblk)
    base = -(-base // 16) * 16
    blks = []
    off = 0
    while off < C:
        sz = min(base, C - off)
        blks.append((off, sz))
        off += sz
    return blks


def build_nc(C):
    assert C % 16 == 0 and C <= T
    blks = _blocks(C)

    nc = bacc.Bacc(
        "TRN2",
        target_bir_lowering=False,
        debug=False,
        num_devices=NCORES,
    )

    xgs = [
        nc.dram_tensor(f"xg{i}", [128, HC, sz], f16, kind="ExternalInput")
        for i, (_, sz) in enumerate(blks)
    ]
    cb = nc.dram_tensor("cb", [C], f32, kind="ExternalInput")
    w1p = nc.dram_tensor("w1p", [FC, 128, HC, 128], f16, kind="ExternalInput")
    w3p = nc.dram_tensor("w3p", [FC, 128, HC, 128], f16, kind="ExternalInput")
    w2p = nc.dram_tensor("w2p", [HC, 2, 128, FC // 2, 128], f16, kind="ExternalInput")
    out = nc.dram_tensor("out", [H, C], f32, kind="ExternalOutput")

    AF = mybir.ActivationFunctionType

    with tile.TileContext(nc) as tc:
        with (
            tc.tile_pool(name="big", bufs=1) as big,
            tc.tile_pool(name="singles", bufs=1) as singles,
            tc.tile_pool(name="wpool", bufs=6) as wpool,
            tc.tile_pool(name="w2pool", bufs=1) as w2pool,
            tc.tile_pool(name="evict", bufs=3) as evict,
            tc.tile_pool(name="psum_w", bufs=2, space="PSUM") as psum_w,
            tc.tile_pool(name="psum_gu", bufs=2, space="PSUM") as psum_gu,
            tc.tile_pool(name="psum_o", bufs=2, space="PSUM") as psum_o,
        ):
            # ---- PE warmup: ramp the p-state while inputs stream in ----
            warm = singles.tile([128, 512], f16, tag="warm")
            nc.vector.memset(warm, 0.0)
            for _ in range(NWARM):
                wps = psum_w.tile([128, 512], f32, tag="wm")
                nc.tensor.matmul(wps, lhsT=warm[:, :128], rhs=warm, start=True, stop=True)

            # ---- input loads ----
            # SP ring: x block 0, then the just-in-time w1/w3 stream.
            # ACT ring: x block 1+, cb, then the whole-w2 preload.
            xb = []
            for i, (_, sz) in enumerate(blks):
                t = singles.tile([128, HC, sz], f16, tag=f"xb{i}")
                eng = nc.sync if i == 0 else nc.scalar
                eng.dma_start(out=t, in_=xgs[i].ap())
                xb.append(t)
            cb_b = singles.tile([128, C], f32, tag="cb")
            cb_src = bass.AP(tensor=cb.ap().tensor, offset=0, ap=[[0, 128], [1, C]])
            nc.scalar.dma_start(out=cb_b, in_=cb_src)

            # ---- phase 1: gus = gelu(w1 @ x) * (w3 @ x), [F-part, tokens] ----
            # The w2 preload is paced through the f-loop (one 512KB chunk per
            # two f-iters on the ACT ring) so its burst can't starve the
            # just-in-time w1/w3 stream of DMA-engine bandwidth.
            w2sb = {}
            w2chunks = [(h, hv) for h in range(HC) for hv in range(2)]
            gus = big.tile([128, FC, C], f16)
            for f in range(FC):
                w1f = wpool.tile([128, HC, 128], f16, tag="w1f")
                nc.sync.dma_start(out=w1f, in_=w1p.ap()[f])
                w3f = wpool.tile([128, HC, 128], f16, tag="w3f")
                nc.sync.dma_start(out=w3f, in_=w3p.ap()[f])
                if f % 2 == 0 and f // 2 < len(w2chunks):
                    h2, hv2 = w2chunks[f // 2]
                    t = w2pool.tile([128, FC // 2, 128], f16, tag=f"w2_{h2}_{hv2}")
                    nc.scalar.dma_start(out=t, in_=w2p.ap()[h2, hv2])
                    w2sb[h2, hv2] = t
                for bi, (off, sz) in enumerate(blks):
                    g_ps = psum_gu.tile([128, 512], f32, tag="g")
                    for h in range(HC):
                        nc.tensor.matmul(
                            g_ps[:, :sz],
                            lhsT=w1f[:, h, :],
                            rhs=xb[bi][:, h, :],
                            start=(h == 0),
                            stop=(h == HC - 1),
                        )
                    u_ps = psum_gu.tile([128, 512], f32, tag="u")
                    for h in range(HC):
                        nc.tensor.matmul(
                            u_ps[:, :sz],
                            lhsT=w3f[:, h, :],
                            rhs=xb[bi][:, h, :],
                            start=(h == 0),
                            stop=(h == HC - 1),
                        )
                    gs = evict.tile([128, 512], f32, tag="gs")
                    nc.scalar.activation(gs[:, :sz], g_ps[:, :sz], AF.Gelu)
                    nc.vector.tensor_mul(
                        gus[:, f, off : off + sz], gs[:, :sz], u_ps[:, :sz]
                    )

            # ---- phase 2: out[h,:] = (w2 @ gus) * cb (w2 already in SBUF) ----
            for h in range(HC):
                for off, sz in blks:
                    o_ps = psum_o.tile([128, 512], f32, tag="o")
                    for f in range(FC):
                        nc.tensor.matmul(
                            o_ps[:, :sz],
                            lhsT=w2sb[h, f // (FC // 2)][:, f % (FC // 2), :],
                            rhs=gus[:, f, off : off + sz],
                            start=(f == 0),
                            stop=(f == FC - 1),
                        )
                    o_sb = evict.tile([128, 512], f32, tag="osb")
                    nc.vector.tensor_mul(
                        o_sb[:, :sz], o_ps[:, :sz], cb_b[:, off : off + sz]
                    )
                    nc.sync.dma_start(
                        out=out.ap()[ts(h, 128), off : off + sz], in_=o_sb[:, :sz]
                    )
    nc.finalize()
    return nc


def _route(hidden_states, gate_w):
    """Exact fp32 router matching the reference: softcap -> softmax -> top2."""
    hs = hidden_states.astype(np.float32)
    logits = hs @ gate_w.T.astype(np.float32)
    logits = 30.0 * np.tanh(logits / 30.0)
    lmax = logits.max(axis=-1, keepdims=True)
    p = np.exp(logits - lmax)
    probs = p / p.sum(axis=-1, keepdims=True)
    idx = np.argsort(-probs, axis=-1, kind="stable")[:, :2]
    vals = np.take_along_axis(probs, idx, axis=-1)
    return idx, vals


def _prep_weights(w1, w2, w3):
    """Pre-transpose expert weights into contiguous SBUF tile layouts."""
    wmaps = []
    for e in range(NCORES):
        # w1p[f, p, c, m] = w1[e][f*128+m, c*128+p]
        w1p = np.ascontiguousarray(
            w1[e].astype(np.float16).reshape(FC, 128, HC, 128).transpose(0, 3, 2, 1)
        )
        w3p = np.ascontiguousarray(
            w3[e].astype(np.float16).reshape(FC, 128, HC, 128).transpose(0, 3, 2, 1)
        )
        # w2p[h, hv, p, c, m] = w2[e][h*128+m, hv*F/2 + c*128+p]
        w2p = np.ascontiguousarray(
            w2[e]
            .astype(np.float16)
            .reshape(HC, 128, 2, FC // 2, 128)
            .transpose(0, 2, 4, 3, 1)
        )
        wmaps.append({"w1p": w1p, "w3p": w3p, "w2p": w2p})
    return wmaps


def kernel(hidden_states, gate_w, w1, w2, w3, trace=False):
    hidden_states = np.asarray(hidden_states, dtype=np.float32)
    gate_w = np.asarray(gate_w, dtype=np.float32)
    w1 = np.asarray(w1, dtype=np.float32)
    w2 = np.asarray(w2, dtype=np.float32)
    w3 = np.asarray(w3, dtype=np.float32)

    idx, vals = _route(hidden_states, gate_w)
    toks = []
    cvals = []
    for e in range(E):
        hit = idx == e                                     # [T, 2]
        tok_e = np.where(hit.any(axis=1))[0]
        toks.append(tok_e)
        cvals.append((vals * hit)[tok_e].sum(axis=1).astype(np.float32))
    nmax = max(len(t) for t in toks)
    C = max(16, -(-nmax // 16) * 16)
    blks = _blocks(C)

    if C not in _CACHE:
        _CACHE[C] = build_nc(C)
    nc = _CACHE[C]

    xT16 = np.ascontiguousarray(hidden_states.T).astype(np.float16)  # [H, T]
    wmaps = _prep_weights(w1, w2, w3)
    in_maps = []
    for e in range(NCORES):
        n_e = len(toks[e])
        xg = np.zeros((128, HC, C), dtype=np.float16)
        # xg[p, c, :n] = x[c*128+p, toks]
        xg[:, :, :n_e] = xT16.reshape(HC, 128, T)[:, :, toks[e]].transpose(1, 0, 2)
        cbv = np.zeros((C,), dtype=np.float32)
        cbv[:n_e] = cvals[e]
        m = {"cb": cbv, **wmaps[e]}
        for i, (off, sz) in enumerate(blks):
            m[f"xg{i}"] = np.ascontiguousarray(xg[:, :, off : off + sz])
        in_maps.append(m)

    res = run_bass_kernel_spmd(nc, in_maps, core_ids=list(range(NCORES)), trace=trace)
    out = np.zeros((T, H), dtype=np.float32)
    for e in range(NCORES):
        n_e = len(toks[e])
        out[toks[e]] += res.results[e]["out"][:, :n_e].T
    _CACHE["last_results"] = res
    return out


if __name__ == "__main__":
    rng = np.random.default_rng(0)
    hs = rng.standard_normal((T, H), dtype=np.float32)
    gw = (rng.standard_normal((E, H)) * 0.02).astype(np.float32)
    w1 = (rng.standard_normal((E, F, H)) * 0.02).astype(np.float32)
    w2 = (rng.standard_normal((E, H, F)) * 0.02).astype(np.float32)
    w3 = (rng.standard_normal((E, F, H)) * 0.02).astype(np.float32)
    out = kernel(hs, gw, w1, w2, w3)
    print("out", out.shape, out.dtype, np.abs(out).max())
